# revision 22
# baseline (speedup 1.0000x reference)
"""KVGather Trainium2 kernel.

Problem: out[n, i, k] = r_weight[n, i, k] * kv[n, r_idx[n, i, k]]
  r_idx:    (16, 64, 8)  int64, values in [0, 64)
  r_weight: (16, 64, 8)  float32
  kv:       (16, 64, 64, 128) float32
  out:      (16, 64, 8, 64, 128) float32

Strategy: data-parallel over batch n across 8 NeuronCores (2 batches/core).
Per core the output write dominates traffic; the rel-err budget (2e-2) is
spent to shrink it:
  - Device computes/stores the output in bf16 (~2^-9 rel err at every
    magnitude); the host casts back to f32.  Store traffic: 16.8 MB/core
    instead of 33.5 MB.
  - kv is kept in bf16 (~2^-9 rel): total ~0.4% worst-case error.
  - The gather runs as one-hot matmuls.  Contraction depth is only 64
    (regions), so the 128x128 PE array is split into FOUR concurrent
    64x64 tiles via tile_position: row half = batch (batch 0 regions on
    partitions 0..63, batch 1 on 64..127), column half = slot group.
    Four matmuls stream simultaneously => ~4x column throughput.
  - DVE/ACT alternate draining PSUM -> bf16 staging fused with the
    f32 weight multiply; per-batch [128,1024] PSUM tiles (2 banks x
    2 bufs x 2 tags = all 8 banks) keep the WAR rotation fine-grained.
  - kv chunk 0 is the first DMA issued so the PE starts early; stores
    fire per f-quarter (16 stores of 1 MB, 8KB rows -- already at the
    ~26 GB/s per-engine packet-rate plateau) so the store stream starts
    early and the tail is short.  All DMA issues from the idle sync
    queue.

Layout per core (supertile st = 0..3 covers slots [st*128,(st+1)*128) of
BOTH batches):
  psum_b[64j+p, :] = sum_r S[b*64+r, st*2+j, p] * kv[b*64+r, :]
  stage[p, fq, b, f2] = psum_b[p, fq*2048+f2] * w[p, st, b]   (bf16)
  out_d[st, fq, p, b, f2] = out[batch b, slot st*128+p, fq*2048+f2]
"""

import sys

for _p in ("/opt/trn_rl_repo",):
    if _p not in sys.path:
        sys.path.insert(0, _p)

import numpy as np
import ml_dtypes

from concourse import bass, bacc, tile
from concourse import mybir
from concourse.bass_utils import run_bass_kernel_spmd

# Problem constants (hardcoded per contract)
N, P2, TOPK, W2, C_KV = 16, 64, 8, 64, 128
N_CORES = 8
B = N // N_CORES            # batches per core = 2
SLOTS = P2 * TOPK           # 512 output slots per batch
F = W2 * C_KV               # 8192 elements per region
ST = 4                      # supertiles; each = 128 slots x 2 batches
FC = 8                      # kv f-dim split for load/compute overlap
F_PER_FC = F // FC          # 1024
TP = F // 1024              # 1024-wide f-pairs per supertile

_cached = {}


def _build_program():
    """Build the (input-independent) Bass program once."""
    if "nc" in _cached:
        return _cached["nc"]

    bf16 = mybir.dt.bfloat16
    f32 = mybir.dt.float32

    nc = bacc.Bacc()

    # kv plane: partition p = (batch p//64, region p%64); free (fc, elem).
    kv_d = nc.dram_tensor("kv", [128, FC, F_PER_FC], bf16, kind="ExternalInput")
    # Selection matrices: s_d[b*64+r, st*2+j, c] = 1.0 iff region r is
    # routed to batch b's slot st*128 + 64*j + c.
    s_d = nc.dram_tensor("sel", [128, ST * 2, 64], bf16, kind="ExternalInput")
    # w_d[p, st, b] = f32 weight of batch b's slot st*128 + p.
    w_d = nc.dram_tensor("wgt", [128, ST, B], f32, kind="ExternalInput")
    # out_d[st, fq, p, b, f2]: weighted gather of batch b's slot st*128+p,
    # f range fq*2048 + f2.
    out_d = nc.dram_tensor("out", [ST, 4, 128, B, F // 4], bf16, kind="ExternalOutput")

    with tile.TileContext(nc) as tc:
        with (
            tc.tile_pool(name="const", bufs=1) as const_pool,
            tc.tile_pool(name="kv", bufs=1) as kv_pool,
            tc.tile_pool(name="stage", bufs=4) as stage_pool,
            tc.tile_pool(name="psum", bufs=2, space=bass.MemorySpace.PSUM) as psum_pool,
        ):
            s_sb = const_pool.tile([128, ST * 2, 64], bf16, tag="sel")
            w_sb = const_pool.tile([128, ST, B], f32, tag="wgt")
            kv_sb = kv_pool.tile([128, FC, F_PER_FC], bf16, tag="kv")

            # Descriptor issue costs ~640ns per dma_start and serializes
            # on the issuing (sync) queue, so issue the pieces that gate
            # the first LDWEIGHTS/matmul first: supertile-0's sel slice,
            # then kv chunk 0.
            nc.sync.dma_start(out=s_sb[:, 0:2], in_=s_d[:, 0:2])
            nc.sync.dma_start(out=kv_sb[:, 0], in_=kv_d[:, 0])
            nc.sync.dma_start(out=s_sb[:, 2:], in_=s_d[:, 2:])
            nc.sync.dma_start(out=kv_sb[:, 1], in_=kv_d[:, 1])
            nc.sync.dma_start(out=w_sb[:], in_=w_d[:])
            for fc in range(2, FC):
                nc.sync.dma_start(out=kv_sb[:, fc], in_=kv_d[:, fc])

            di = 0
            for st in range(ST):
                stage = stage_pool.tile([128, 4, B, F // 4], bf16, tag="stage")
                for tp in range(TP):
                    fq, tq = divmod(tp, 2)
                    ps = [
                        psum_pool.tile([128, 1024], f32, tag=f"ps{b}", name=f"ps{b}")
                        for b in range(B)
                    ]
                    for h in range(2):
                        off = h * 512
                        for b in range(B):
                            for j in range(2):
                                nc.tensor.matmul(
                                    ps[b][j * 64 : (j + 1) * 64, off : off + 512],
                                    s_sb[b * 64 : (b + 1) * 64, st * 2 + j, :],
                                    kv_sb[b * 64 : (b + 1) * 64, tp, off : off + 512],
                                    start=True,
                                    stop=True,
                                )
                    for b in range(B):
                        sl = stage[:, fq, b, tq * 1024 : (tq + 1) * 1024]
                        if di % 2 == 0:
                            nc.vector.tensor_mul(
                                sl,
                                ps[b][:],
                                w_sb[:, st, b : b + 1].broadcast_to([128, 1024]),
                            )
                        else:
                            nc.scalar.activation(
                                sl,
                                ps[b][:],
                                mybir.ActivationFunctionType.Copy,
                                scale=w_sb[:, st, b : b + 1],
                            )
                        di += 1
                    if tq == 1:
                        # Store the finished f-quarter: 128 rows x 8KB (1 MB).
                        nc.sync.dma_start(out=out_d[st, fq], in_=stage[:, fq])

    nc.compile()
    _cached["nc"] = nc
    return nc


def _prep_inputs(r_idx, r_weight, kv):
    """Shard + transform host inputs into per-core in_maps."""
    bf16 = ml_dtypes.bfloat16
    r_idx = np.asarray(r_idx).astype(np.int64)
    r_weight = np.asarray(r_weight).astype(np.float32)
    kv = np.asarray(kv).astype(np.float32).reshape(N, P2, F)

    kv_bf = kv.astype(bf16)

    in_maps = []
    for m in range(N_CORES):
        bsl = slice(m * B, (m + 1) * B)
        idx = r_idx[bsl].reshape(B, SLOTS)        # [2, 512] region ids
        wgt = r_weight[bsl].reshape(B, SLOTS)     # [2, 512] f32

        plane = kv_bf[bsl].reshape(128, FC, F_PER_FC)

        S = np.zeros((128, ST * 2, 64), dtype=bf16)
        W = np.zeros((128, ST, B), dtype=np.float32)
        cols = np.arange(64)
        for st in range(ST):
            for b in range(B):
                for j in range(2):
                    slots = st * 128 + 64 * j + cols
                    r = idx[b, slots]
                    S[b * 64 + r, st * 2 + j, cols] = 1.0
                W[:, st, b] = wgt[b, st * 128 + np.arange(128)]

        in_maps.append(
            {"kv": np.ascontiguousarray(plane), "sel": S, "wgt": W}
        )
    return in_maps


def _ensure_ntff_hook():
    """The agent image's antenv lacks axon_hooks, so the boot-time NTFF
    hook registration silently no-ops. Recreate the module and register
    the ctypes hook so trace=True yields exec_time_ns."""
    import types
    import antenv

    if "antenv.axon_hooks" in sys.modules:
        return
    mod = types.ModuleType("antenv.axon_hooks")
    _state = {"hook": None}
    mod.set_axon_ntff_profile_hook = lambda h: _state.__setitem__("hook", h)
    mod.get_axon_ntff_profile_hook = lambda: _state["hook"]
    sys.modules["antenv.axon_hooks"] = mod
    antenv.axon_hooks = mod
    try:
        if "/root/.axon_site" not in sys.path:
            sys.path.insert(0, "/root/.axon_site")
        from trn_agent_boot.trn_boot import _ntff_profile_via_ctypes

        hook = _ntff_profile_via_ctypes("/opt/axon/libaxon_pjrt.so")
        if hook is not None:
            mod.set_axon_ntff_profile_hook(hook)
    except Exception:
        pass


def kernel(r_idx, r_weight, kv, _trace=False, _trace_kwargs=None):
    if _trace:
        _ensure_ntff_hook()
    nc = _build_program()
    in_maps = _prep_inputs(r_idx, r_weight, kv)
    res = run_bass_kernel_spmd(
        nc,
        in_maps,
        core_ids=list(range(N_CORES)),
        trace=_trace,
        **(_trace_kwargs or {}),
    )
    out = np.empty((N, P2, TOPK, W2, C_KV), dtype=np.float32)
    for m in range(N_CORES):
        o = res.results[m]["out"]  # [ST, 4, 128, B, F//4] bf16
        o = np.asarray(o).astype(np.float32)
        # (st, fq, p, b, f2) -> (b, st*128+p, fq*2048+f2) = (batch, slot, f)
        o = np.transpose(o, (3, 0, 2, 1, 4)).reshape(B, SLOTS, F)
        out[m * B : (m + 1) * B] = o.reshape(B, P2, TOPK, W2, C_KV)
    if _trace:
        return out, res
    return out


if __name__ == "__main__":
    rng = np.random.default_rng(0)
    r_idx = rng.integers(0, P2, (N, P2, TOPK)).astype(np.int64)
    r_weight = rng.random((N, P2, TOPK), dtype=np.float32)
    kv = rng.standard_normal((N, P2, W2, C_KV), dtype=np.float32)
    out = kernel(r_idx, r_weight, kv)
    # local reference
    bidx = np.arange(N)[:, None, None]
    exp = r_weight[..., None, None] * kv[bidx, r_idx]
    err = np.abs(out - exp).max() / (np.abs(exp).max() + 1e-30)
    print("abs-rel err:", err)


# revision 23
# speedup vs baseline: 1.0276x; 1.0276x over previous
"""KVGather Trainium2 kernel.

Problem: out[n, i, k] = r_weight[n, i, k] * kv[n, r_idx[n, i, k]]
  r_idx:    (16, 64, 8)  int64, values in [0, 64)
  r_weight: (16, 64, 8)  float32
  kv:       (16, 64, 64, 128) float32
  out:      (16, 64, 8, 64, 128) float32

Strategy: data-parallel over batch n across 8 NeuronCores (2 batches/core).
Per core the output write dominates traffic; the rel-err budget (2e-2) is
spent to shrink it:
  - Device computes/stores the output in bf16 (~2^-9 rel err at every
    magnitude); the host casts back to f32.  Store traffic: 16.8 MB/core
    instead of 33.5 MB.
  - kv is kept in bf16 (~2^-9 rel): total ~0.4% worst-case error.
  - The gather runs as one-hot matmuls.  Contraction depth is only 64
    (regions), so the 128x128 PE array is split into FOUR concurrent
    64x64 tiles via tile_position: row half = batch (batch 0 regions on
    partitions 0..63, batch 1 on 64..127), column half = slot group.
    Four matmuls stream simultaneously => ~4x column throughput.
  - DVE/ACT alternate draining PSUM -> bf16 staging fused with the
    f32 weight multiply; per-batch [128,1024] PSUM tiles (2 banks x
    2 bufs x 2 tags = all 8 banks) keep the WAR rotation fine-grained.
  - kv chunk 0 is the first DMA issued so the PE starts early; stores
    fire per f-quarter (16 stores of 1 MB, 8KB rows -- already at the
    ~26 GB/s per-engine packet-rate plateau) so the store stream starts
    early and the tail is short.  All DMA issues from the idle sync
    queue.

Layout per core (supertile st = 0..3 covers slots [st*128,(st+1)*128) of
BOTH batches):
  psum_b[64j+p, :] = sum_r S[b*64+r, st*2+j, p] * kv[b*64+r, :]
  stage[p, fq, b, f2] = psum_b[p, fq*2048+f2] * w[p, st, b]   (bf16)
  out_d[st, fq, p, b, f2] = out[batch b, slot st*128+p, fq*2048+f2]
"""

import sys

for _p in ("/opt/trn_rl_repo",):
    if _p not in sys.path:
        sys.path.insert(0, _p)

import numpy as np
import ml_dtypes

from concourse import bass, bacc, tile
from concourse import mybir
from concourse.bass_utils import run_bass_kernel_spmd

# Problem constants (hardcoded per contract)
N, P2, TOPK, W2, C_KV = 16, 64, 8, 64, 128
N_CORES = 8
B = N // N_CORES            # batches per core = 2
SLOTS = P2 * TOPK           # 512 output slots per batch
F = W2 * C_KV               # 8192 elements per region
ST = 4                      # supertiles; each = 128 slots x 2 batches
FC = 8                      # kv f-dim split for load/compute overlap
F_PER_FC = F // FC          # 1024
TP = F // 1024              # 1024-wide f-pairs per supertile

_cached = {}


def _build_program():
    """Build the (input-independent) Bass program once."""
    if "nc" in _cached:
        return _cached["nc"]

    bf16 = mybir.dt.bfloat16
    f32 = mybir.dt.float32

    nc = bacc.Bacc()

    # kv plane: partition p = (batch p//64, region p%64); free (fc, elem).
    kv_d = nc.dram_tensor("kv", [128, FC, F_PER_FC], bf16, kind="ExternalInput")
    # Selection matrices: s_d[b*64+r, st*2+j, c] = 1.0 iff region r is
    # routed to batch b's slot st*128 + 64*j + c.
    s_d = nc.dram_tensor("sel", [128, ST * 2, 64], bf16, kind="ExternalInput")
    # w_d[p, st, b] = f32 weight of batch b's slot st*128 + p.
    w_d = nc.dram_tensor("wgt", [128, ST, B], f32, kind="ExternalInput")
    # out_d[st, fq, p, b, f2]: weighted gather of batch b's slot st*128+p,
    # f range fq*2048 + f2.
    out_d = nc.dram_tensor("out", [ST, 4, 128, B, F // 4], bf16, kind="ExternalOutput")

    with tile.TileContext(nc) as tc:
        with (
            tc.tile_pool(name="const", bufs=1) as const_pool,
            tc.tile_pool(name="kv", bufs=1) as kv_pool,
            tc.tile_pool(name="stage", bufs=4) as stage_pool,
            tc.tile_pool(name="psum", bufs=2, space=bass.MemorySpace.PSUM) as psum_pool,
        ):
            s_sb = const_pool.tile([128, ST * 2, 64], bf16, tag="sel")
            w_sb = const_pool.tile([128, ST, B], f32, tag="wgt")
            kv_sb = kv_pool.tile([128, FC, F_PER_FC], bf16, tag="kv")

            # kv chunk 0 first: it gates the first matmul.  All DMA is
            # issued from the (otherwise idle) sync queue.  (Measured:
            # splitting issues across sync+gpsimd, or issuing a sel
            # slice first, is ~1us SLOWER in uncontended runs.)
            nc.sync.dma_start(out=kv_sb[:, 0], in_=kv_d[:, 0])
            nc.sync.dma_start(out=s_sb[:], in_=s_d[:])
            nc.sync.dma_start(out=w_sb[:], in_=w_d[:])
            for fc in range(1, FC):
                nc.sync.dma_start(out=kv_sb[:, fc], in_=kv_d[:, fc])

            di = 0
            for st in range(ST):
                stage = stage_pool.tile([128, 4, B, F // 4], bf16, tag="stage")
                for tp in range(TP):
                    fq, tq = divmod(tp, 2)
                    ps = [
                        psum_pool.tile([128, 1024], f32, tag=f"ps{b}", name=f"ps{b}")
                        for b in range(B)
                    ]
                    for h in range(2):
                        off = h * 512
                        for b in range(B):
                            for j in range(2):
                                nc.tensor.matmul(
                                    ps[b][j * 64 : (j + 1) * 64, off : off + 512],
                                    s_sb[b * 64 : (b + 1) * 64, st * 2 + j, :],
                                    kv_sb[b * 64 : (b + 1) * 64, tp, off : off + 512],
                                    start=True,
                                    stop=True,
                                )
                    for b in range(B):
                        sl = stage[:, fq, b, tq * 1024 : (tq + 1) * 1024]
                        if di % 2 == 0:
                            nc.vector.tensor_mul(
                                sl,
                                ps[b][:],
                                w_sb[:, st, b : b + 1].broadcast_to([128, 1024]),
                            )
                        else:
                            nc.scalar.activation(
                                sl,
                                ps[b][:],
                                mybir.ActivationFunctionType.Copy,
                                scale=w_sb[:, st, b : b + 1],
                            )
                        di += 1
                    if tq == 1:
                        # Store the finished f-quarter: 128 rows x 8KB (1 MB).
                        nc.sync.dma_start(out=out_d[st, fq], in_=stage[:, fq])

    nc.compile()
    _cached["nc"] = nc
    return nc


def _prep_inputs(r_idx, r_weight, kv):
    """Shard + transform host inputs into per-core in_maps."""
    bf16 = ml_dtypes.bfloat16
    r_idx = np.asarray(r_idx).astype(np.int64)
    r_weight = np.asarray(r_weight).astype(np.float32)
    kv = np.asarray(kv).astype(np.float32).reshape(N, P2, F)

    kv_bf = kv.astype(bf16)

    in_maps = []
    for m in range(N_CORES):
        bsl = slice(m * B, (m + 1) * B)
        idx = r_idx[bsl].reshape(B, SLOTS)        # [2, 512] region ids
        wgt = r_weight[bsl].reshape(B, SLOTS)     # [2, 512] f32

        plane = kv_bf[bsl].reshape(128, FC, F_PER_FC)

        S = np.zeros((128, ST * 2, 64), dtype=bf16)
        W = np.zeros((128, ST, B), dtype=np.float32)
        cols = np.arange(64)
        for st in range(ST):
            for b in range(B):
                for j in range(2):
                    slots = st * 128 + 64 * j + cols
                    r = idx[b, slots]
                    S[b * 64 + r, st * 2 + j, cols] = 1.0
                W[:, st, b] = wgt[b, st * 128 + np.arange(128)]

        in_maps.append(
            {"kv": np.ascontiguousarray(plane), "sel": S, "wgt": W}
        )
    return in_maps


def _ensure_ntff_hook():
    """The agent image's antenv lacks axon_hooks, so the boot-time NTFF
    hook registration silently no-ops. Recreate the module and register
    the ctypes hook so trace=True yields exec_time_ns."""
    import types
    import antenv

    if "antenv.axon_hooks" in sys.modules:
        return
    mod = types.ModuleType("antenv.axon_hooks")
    _state = {"hook": None}
    mod.set_axon_ntff_profile_hook = lambda h: _state.__setitem__("hook", h)
    mod.get_axon_ntff_profile_hook = lambda: _state["hook"]
    sys.modules["antenv.axon_hooks"] = mod
    antenv.axon_hooks = mod
    try:
        if "/root/.axon_site" not in sys.path:
            sys.path.insert(0, "/root/.axon_site")
        from trn_agent_boot.trn_boot import _ntff_profile_via_ctypes

        hook = _ntff_profile_via_ctypes("/opt/axon/libaxon_pjrt.so")
        if hook is not None:
            mod.set_axon_ntff_profile_hook(hook)
    except Exception:
        pass


def kernel(r_idx, r_weight, kv, _trace=False, _trace_kwargs=None):
    if _trace:
        _ensure_ntff_hook()
    nc = _build_program()
    in_maps = _prep_inputs(r_idx, r_weight, kv)
    res = run_bass_kernel_spmd(
        nc,
        in_maps,
        core_ids=list(range(N_CORES)),
        trace=_trace,
        **(_trace_kwargs or {}),
    )
    out = np.empty((N, P2, TOPK, W2, C_KV), dtype=np.float32)
    for m in range(N_CORES):
        o = res.results[m]["out"]  # [ST, 4, 128, B, F//4] bf16
        o = np.asarray(o).astype(np.float32)
        # (st, fq, p, b, f2) -> (b, st*128+p, fq*2048+f2) = (batch, slot, f)
        o = np.transpose(o, (3, 0, 2, 1, 4)).reshape(B, SLOTS, F)
        out[m * B : (m + 1) * B] = o.reshape(B, P2, TOPK, W2, C_KV)
    if _trace:
        return out, res
    return out


if __name__ == "__main__":
    rng = np.random.default_rng(0)
    r_idx = rng.integers(0, P2, (N, P2, TOPK)).astype(np.int64)
    r_weight = rng.random((N, P2, TOPK), dtype=np.float32)
    kv = rng.standard_normal((N, P2, W2, C_KV), dtype=np.float32)
    out = kernel(r_idx, r_weight, kv)
    # local reference
    bidx = np.arange(N)[:, None, None]
    exp = r_weight[..., None, None] * kv[bidx, r_idx]
    err = np.abs(out - exp).max() / (np.abs(exp).max() + 1e-30)
    print("abs-rel err:", err)


# revision 24
# speedup vs baseline: 1.1104x; 1.0806x over previous
"""KVGather Trainium2 kernel.

Problem: out[n, i, k] = r_weight[n, i, k] * kv[n, r_idx[n, i, k]]
  r_idx:    (16, 64, 8)  int64, values in [0, 64)
  r_weight: (16, 64, 8)  float32
  kv:       (16, 64, 64, 128) float32
  out:      (16, 64, 8, 64, 128) float32

Strategy: data-parallel over batch n across 8 NeuronCores (2 batches/core).
Per core the output write dominates traffic; the rel-err budget (2e-2) is
spent to shrink it:
  - Device computes/stores the output in bf16 (~2^-9 rel err at every
    magnitude); the host casts back to f32.  Store traffic: 16.8 MB/core
    instead of 33.5 MB.
  - kv is kept in bf16 (~2^-9 rel): total ~0.4% worst-case error.
  - The gather runs as one-hot matmuls.  Contraction depth is only 64
    (regions), so the 128x128 PE array is split into FOUR concurrent
    64x64 tiles via tile_position: row half = batch (batch 0 regions on
    partitions 0..63, batch 1 on 64..127), column half = slot group.
    Four matmuls stream simultaneously => ~4x column throughput.
  - DVE/ACT alternate draining PSUM -> bf16 staging fused with the
    f32 weight multiply; per-batch [128,1024] PSUM tiles (2 banks x
    2 bufs x 2 tags = all 8 banks) keep the WAR rotation fine-grained.
  - kv chunk 0 is the first DMA issued so the PE starts early; stores
    fire per f-quarter (16 stores of 1 MB, 8KB rows -- already at the
    ~26 GB/s per-engine packet-rate plateau) so the store stream starts
    early and the tail is short.  All DMA issues from the idle sync
    queue.

Layout per core (supertile st = 0..3 covers slots [st*128,(st+1)*128) of
BOTH batches):
  psum_b[64j+p, :] = sum_r S[b*64+r, st*2+j, p] * kv[b*64+r, :]
  stage[p, fq, b, f2] = psum_b[p, fq*2048+f2] * w[p, st, b]   (bf16)
  out_d[st, fq, p, b, f2] = out[batch b, slot st*128+p, fq*2048+f2]
"""

import sys

for _p in ("/opt/trn_rl_repo",):
    if _p not in sys.path:
        sys.path.insert(0, _p)

import numpy as np
import ml_dtypes

from concourse import bass, bacc, tile
from concourse import mybir
from concourse.bass_utils import run_bass_kernel_spmd

# Problem constants (hardcoded per contract)
N, P2, TOPK, W2, C_KV = 16, 64, 8, 64, 128
N_CORES = 8
B = N // N_CORES            # batches per core = 2
SLOTS = P2 * TOPK           # 512 output slots per batch
F = W2 * C_KV               # 8192 elements per region
ST = 4                      # supertiles; each = 128 slots x 2 batches
FC = 8                      # kv f-dim split for load/compute overlap
F_PER_FC = F // FC          # 1024
TP = F // 1024              # 1024-wide f-pairs per supertile

_cached = {}


def _build_program():
    """Build the (input-independent) Bass program once."""
    if "nc" in _cached:
        return _cached["nc"]

    bf16 = mybir.dt.bfloat16
    f32 = mybir.dt.float32

    nc = bacc.Bacc()

    # kv plane: partition p = (batch p//64, region p%64); free (fc, elem).
    kv_d = nc.dram_tensor("kv", [128, FC, F_PER_FC], bf16, kind="ExternalInput")
    # Selection matrices: s_d[b*64+r, st*2+j, c] = 1.0 iff region r is
    # routed to batch b's slot st*128 + 64*j + c.
    s_d = nc.dram_tensor("sel", [128, ST * 2, 64], bf16, kind="ExternalInput")
    # w_d[p, st, b] = f32 weight of batch b's slot st*128 + p.
    w_d = nc.dram_tensor("wgt", [128, ST, B], f32, kind="ExternalInput")
    # out_d[st, fq, p, b, f2]: weighted gather of batch b's slot st*128+p,
    # f range fq*2048 + f2.
    out_d = nc.dram_tensor("out", [ST, 4, 128, B, F // 4], bf16, kind="ExternalOutput")

    with tile.TileContext(nc) as tc:
        with (
            tc.tile_pool(name="const", bufs=1) as const_pool,
            tc.tile_pool(name="kv", bufs=1) as kv_pool,
            tc.tile_pool(name="stage", bufs=4) as stage_pool,
            tc.tile_pool(name="psum", bufs=2, space=bass.MemorySpace.PSUM) as psum_pool,
        ):
            s_sb = const_pool.tile([128, ST * 2, 64], bf16, tag="sel")
            w_sb = const_pool.tile([128, ST, B], f32, tag="wgt")
            kv_sb = kv_pool.tile([128, FC, F_PER_FC], bf16, tag="kv")

            # sel then kv chunk 0: together they gate the first
            # ldweights+matmul.  All DMA is issued from the (otherwise
            # idle) sync queue.
            nc.sync.dma_start(out=s_sb[:], in_=s_d[:])
            nc.sync.dma_start(out=kv_sb[:, 0], in_=kv_d[:, 0])
            nc.sync.dma_start(out=w_sb[:], in_=w_d[:])
            for fc in range(1, FC):
                nc.sync.dma_start(out=kv_sb[:, fc], in_=kv_d[:, fc])

            di = 0
            for st in range(ST):
                stage = stage_pool.tile([128, 4, B, F // 4], bf16, tag="stage")
                for tp in range(TP):
                    fq, tq = divmod(tp, 2)
                    ps = [
                        psum_pool.tile([128, 1024], f32, tag=f"ps{b}", name=f"ps{b}")
                        for b in range(B)
                    ]
                    for h in range(2):
                        off = h * 512
                        for b in range(B):
                            for j in range(2):
                                nc.tensor.matmul(
                                    ps[b][j * 64 : (j + 1) * 64, off : off + 512],
                                    s_sb[b * 64 : (b + 1) * 64, st * 2 + j, :],
                                    kv_sb[b * 64 : (b + 1) * 64, tp, off : off + 512],
                                    start=True,
                                    stop=True,
                                )
                    for b in range(B):
                        sl = stage[:, fq, b, tq * 1024 : (tq + 1) * 1024]
                        if di % 2 == 0:
                            nc.vector.tensor_mul(
                                sl,
                                ps[b][:],
                                w_sb[:, st, b : b + 1].broadcast_to([128, 1024]),
                            )
                        else:
                            nc.scalar.activation(
                                sl,
                                ps[b][:],
                                mybir.ActivationFunctionType.Copy,
                                scale=w_sb[:, st, b : b + 1],
                            )
                        di += 1
                    if tq == 1:
                        # Store the finished f-quarter: 128 rows x 8KB (1 MB).
                        nc.sync.dma_start(out=out_d[st, fq], in_=stage[:, fq])

    nc.compile()
    _cached["nc"] = nc
    return nc


def _prep_inputs(r_idx, r_weight, kv):
    """Shard + transform host inputs into per-core in_maps."""
    bf16 = ml_dtypes.bfloat16
    r_idx = np.asarray(r_idx).astype(np.int64)
    r_weight = np.asarray(r_weight).astype(np.float32)
    kv = np.asarray(kv).astype(np.float32).reshape(N, P2, F)

    kv_bf = kv.astype(bf16)

    in_maps = []
    for m in range(N_CORES):
        bsl = slice(m * B, (m + 1) * B)
        idx = r_idx[bsl].reshape(B, SLOTS)        # [2, 512] region ids
        wgt = r_weight[bsl].reshape(B, SLOTS)     # [2, 512] f32

        plane = kv_bf[bsl].reshape(128, FC, F_PER_FC)

        S = np.zeros((128, ST * 2, 64), dtype=bf16)
        W = np.zeros((128, ST, B), dtype=np.float32)
        cols = np.arange(64)
        for st in range(ST):
            for b in range(B):
                for j in range(2):
                    slots = st * 128 + 64 * j + cols
                    r = idx[b, slots]
                    S[b * 64 + r, st * 2 + j, cols] = 1.0
                W[:, st, b] = wgt[b, st * 128 + np.arange(128)]

        in_maps.append(
            {"kv": np.ascontiguousarray(plane), "sel": S, "wgt": W}
        )
    return in_maps


def _ensure_ntff_hook():
    """The agent image's antenv lacks axon_hooks, so the boot-time NTFF
    hook registration silently no-ops. Recreate the module and register
    the ctypes hook so trace=True yields exec_time_ns."""
    import types
    import antenv

    if "antenv.axon_hooks" in sys.modules:
        return
    mod = types.ModuleType("antenv.axon_hooks")
    _state = {"hook": None}
    mod.set_axon_ntff_profile_hook = lambda h: _state.__setitem__("hook", h)
    mod.get_axon_ntff_profile_hook = lambda: _state["hook"]
    sys.modules["antenv.axon_hooks"] = mod
    antenv.axon_hooks = mod
    try:
        if "/root/.axon_site" not in sys.path:
            sys.path.insert(0, "/root/.axon_site")
        from trn_agent_boot.trn_boot import _ntff_profile_via_ctypes

        hook = _ntff_profile_via_ctypes("/opt/axon/libaxon_pjrt.so")
        if hook is not None:
            mod.set_axon_ntff_profile_hook(hook)
    except Exception:
        pass


def kernel(r_idx, r_weight, kv, _trace=False, _trace_kwargs=None):
    if _trace:
        _ensure_ntff_hook()
    nc = _build_program()
    in_maps = _prep_inputs(r_idx, r_weight, kv)
    res = run_bass_kernel_spmd(
        nc,
        in_maps,
        core_ids=list(range(N_CORES)),
        trace=_trace,
        **(_trace_kwargs or {}),
    )
    out = np.empty((N, P2, TOPK, W2, C_KV), dtype=np.float32)
    for m in range(N_CORES):
        o = res.results[m]["out"]  # [ST, 4, 128, B, F//4] bf16
        o = np.asarray(o).astype(np.float32)
        # (st, fq, p, b, f2) -> (b, st*128+p, fq*2048+f2) = (batch, slot, f)
        o = np.transpose(o, (3, 0, 2, 1, 4)).reshape(B, SLOTS, F)
        out[m * B : (m + 1) * B] = o.reshape(B, P2, TOPK, W2, C_KV)
    if _trace:
        return out, res
    return out


if __name__ == "__main__":
    rng = np.random.default_rng(0)
    r_idx = rng.integers(0, P2, (N, P2, TOPK)).astype(np.int64)
    r_weight = rng.random((N, P2, TOPK), dtype=np.float32)
    kv = rng.standard_normal((N, P2, W2, C_KV), dtype=np.float32)
    out = kernel(r_idx, r_weight, kv)
    # local reference
    bidx = np.arange(N)[:, None, None]
    exp = r_weight[..., None, None] * kv[bidx, r_idx]
    err = np.abs(out - exp).max() / (np.abs(exp).max() + 1e-30)
    print("abs-rel err:", err)
